# revision 12
# baseline (speedup 1.0000x reference)
"""Trainium2 Bass kernel for nn_Interpolator — Gaussian-scatter + P2P, v4.

Reference (N=32768 obs, R=2048 sorted ref timesteps, ninp=64, a=50):
    Ks[r,n] = exp(-a(ref_r - t_n)^2)*mask + EPS,  Kc same with 10a
    lam_s = Ks@onehot + EPS, num_s = Ks@(onehot*v), likewise coarse
    lam = lam_s/R; cross = (num_s@rho)/rowsum(lam_s); coarse = num_c/lam_c
    out = concat([lam, cross, coarse-cross], -1)   [1, R, 192]

Algorithm (NUFFT-style Gaussian gridding): scatter each observation onto a
uniform G=128 grid with a narrow Gaussian V (sigma_v = 1.5*dg), so
B_T[h,q] = sum_n V(h - t_n) * comb[n,q] accumulates ALL four segment sums
in one [128,128,128] matmul per 128-obs chunk (comb = [onehot*mask |
onehot*mask*v]).  Both kernels are then recovered exactly by grid-to-ref
matmuls with host-precomputed deconvolved Gaussians Kp (Gaussian*Gaussian
convolution identity; aliasing error ~e^-37).  No transposes needed:
the loop matmul directly produces the [h, q] layout the finish consumes.

Obs axis sharded 8 ways.  The partial B_T [128,128] bf16 (32 KB) is
combined across cores with remote_dma_broadcast (SBUF->SBUF P2P over
RMTV/D2D, SWDGE descriptors prepared after the loop, one trigger) instead
of a CC AllReduce — the ncfw first-call path cost ~55 us in the baseline.
Each core broadcasts its slab into slot j of every peer's receive buffer
(relative XOR addressing makes the same program valid on all 8 cores),
waits for recv_sem == 16 (8 senders x 2), then tree-adds the 8 slabs and
finishes only its own 256 ref columns: 2 grid matmuls, EPS correction,
reciprocals, rho matmul, writes [192, 256]; the host transposes and
concatenates the slices.
"""

import os
import sys

import numpy as np

sys.path.insert(0, "/opt/trn_rl_repo")

import concourse.bass as bass
import concourse.tile as tile
from concourse import bacc, mybir

# The image's antenv package lacks axon_hooks (NTFF profiling registry);
# register one so trace=True can profile HW exec time. Harmless if unused.
try:
    import antenv.axon_hooks  # noqa: F401
except ImportError:
    import types as _types

    _m = _types.ModuleType("antenv.axon_hooks")
    _m._hook = None

    def _set_hook(hook):
        _m._hook = hook

    def _get_hook():
        if _m._hook is None:
            try:
                from trn_agent_boot.trn_boot import _ntff_profile_via_ctypes

                _m._hook = _ntff_profile_via_ctypes("/opt/axon/libaxon_pjrt.so")
            except Exception:
                _m._hook = None
        return _m._hook

    _m.set_axon_ntff_profile_hook = _set_hook
    _m.get_axon_ntff_profile_hook = _get_hook
    sys.modules["antenv.axon_hooks"] = _m
    try:
        import antenv

        antenv.axon_hooks = _m
    except ImportError:
        pass

F32 = mybir.dt.float32
BF16 = mybir.dt.bfloat16
Alu = mybir.AluOpType
Act = mybir.ActivationFunctionType

N = 32768
R = 2048
NI = 64
M = 8
ND = N // M          # 4096 obs per core
P = 128
NCHUNK = ND // P     # 32
G = 128              # scatter grid points
RS = R // M          # 256 ref rows finished per core
EPS = 1e-7
K_SCALE = 10.0

GRID_LO = -0.05
GRID_HI = 1.05
DG = (GRID_HI - GRID_LO) / (G - 1)
SIG_V = 1.5 * DG
BV = 1.0 / (2.0 * SIG_V * SIG_V)


def build_program():
    import contextlib

    nc = bacc.Bacc("TRN2")

    # host-reordered so every DMA is contiguous: comb[p, c, :] = row c*128+p
    comb_in = nc.declare_dram_parameter(
        "comb", [P, NCHUNK, 2 * NI], BF16, isOutput=False
    )
    gb_in = nc.declare_dram_parameter("gb", [P, G], F32, isOutput=False)
    tsc_in = nc.declare_dram_parameter("tsc", [P, NCHUNK], F32, isOutput=False)
    rho_in = nc.declare_dram_parameter("rho", [NI, NI], F32, isOutput=False)
    # corr[0:64] = EPS*(cnt+1); corr[64:128] = EPS*sv  (per-dim EPS pads)
    corr_in = nc.declare_dram_parameter("corr", [P, 1], F32, isOutput=False)
    # per-core deconvolved grid->ref kernels: [G, {smooth,coarse}, RS]
    kp_in = nc.declare_dram_parameter("kp", [G, 2, RS], BF16, isOutput=False)
    # output slice, quantity-major; host transposes to [RS, 192]
    out_t = nc.declare_dram_parameter("out", [3 * NI, RS], F32, isOutput=True)

    recv_sem = nc.alloc_semaphore("p2p_recv")
    send_sem = nc.alloc_semaphore("p2p_send")
    prep_sem = nc.alloc_semaphore("p2p_prep")
    dvec = nc.alloc_semaphore("chain_dve")
    pec = nc.alloc_semaphore("chain_pe")
    actc = nc.alloc_semaphore("chain_act")
    out_sem = nc.alloc_semaphore("fin_out")

    es = contextlib.ExitStack()
    sb = lambda name, shape, dtype: es.enter_context(
        nc.sbuf_tensor(name, shape, dtype)
    )
    ps = lambda name, shape: es.enter_context(
        nc.psum_tensor(name, shape, F32)
    )

    # buffers shared between the tile-scheduled loop and the raw phase, or
    # used only by the raw phase (tiles would stay symbolic there)
    corr_col = sb("corr_col", [P, 1], F32)
    rho_sb = sb("rho_sb", [NI, NI], F32)
    kp_sb = sb("kp_sb", [G, 2, RS], BF16)
    ones_row = sb("ones_row", [1, P], F32)
    ones_col = sb("ones_col", [NI, 1], F32)
    part = sb("part", [P, G], BF16)
    recv = sb("recv", [P, M, G], BF16)
    s4 = sb("s4", [P, 4, G], BF16)
    s2 = sb("s2", [P, 2, G], BF16)
    bred = sb("bred", [P, G], BF16)
    sb_ls = sb("sb_ls", [NI, RS], F32)
    sb_ns = sb("sb_ns", [NI, RS], F32)
    sb_lc = sb("sb_lc", [NI, RS], F32)
    sb_nq = sb("sb_nq", [NI, RS], F32)
    rec = sb("rec", [NI, RS], F32)
    recd = sb("recd", [1, RS], F32)
    lam_out = sb("lam_out", [NI, RS], F32)
    coarse = sb("coarse", [NI, RS], F32)
    dbc = sb("dbc", [NI, RS], F32)
    cross = sb("cross", [NI, RS], F32)
    transient = sb("transient", [NI, RS], F32)

    ip_ls = ps("ip_ls", [NI, RS])
    ip_ns = ps("ip_ns", [NI, RS])
    ip_lc = ps("ip_lc", [NI, RS])
    ip_nq = ps("ip_nq", [NI, RS])
    dps = ps("dps", [1, RS])
    crp = ps("crp", [NI, RS])
    dbp = ps("dbp", [NI, RS])

    with tile.TileContext(nc) as tc:
        with tc.tile_pool(name="consts", bufs=1) as consts:
            # ---------------- constants ----------------
            gb = consts.tile([P, G], F32)
            nc.sync.dma_start(out=gb[:], in_=gb_in[:])
            tsc = consts.tile([P, NCHUNK], F32)
            nc.sync.dma_start(out=tsc[:], in_=tsc_in[:])
            combH = consts.tile([P, NCHUNK, 2 * NI], BF16)
            for q4 in range(4):
                cs = q4 * (NCHUNK // 4)
                ce = cs + NCHUNK // 4
                nc.sync.dma_start(
                    out=combH[:, cs:ce, :], in_=comb_in[:, cs:ce, :]
                )
            nc.sync.dma_start(out=corr_col[:], in_=corr_in[:])
            nc.sync.dma_start(out=rho_sb[:], in_=rho_in[:])
            nc.sync.dma_start(out=kp_sb[:], in_=kp_in[:])
            nc.vector.memset(ones_row[:], 1.0)
            nc.vector.memset(ones_col[:], 1.0)

            # ---------------- main loop (tile-scheduled) ----------------
            with (
                tc.tile_pool(name="acc", bufs=1, space="PSUM") as accpool,
                tc.tile_pool(name="work", bufs=4) as work,
            ):
                acc = accpool.tile([P, G], F32, name="acc", tag="acc")

                for c in range(NCHUNK):
                    d = work.tile([P, G], F32, tag="d")
                    nc.vector.tensor_scalar(
                        out=d[:], in0=gb[:], scalar1=tsc[:, c : c + 1],
                        scalar2=None, op0=Alu.subtract,
                    )
                    d2 = work.tile([P, G], F32, tag="d2")
                    nc.gpsimd.tensor_tensor(
                        out=d2[:], in0=d[:], in1=d[:], op=Alu.mult
                    )
                    kek = work.tile([P, G], BF16, tag="kek")
                    nc.scalar.activation(
                        out=kek[:], in_=d2[:], func=Act.Exp, scale=-BV
                    )
                    nc.tensor.matmul(
                        acc[:, :],
                        kek[:, :],
                        combH[:, c, :],
                        start=(c == 0),
                        stop=(c == NCHUNK - 1),
                    )

                nc.vector.tensor_copy(out=part[:], in_=acc[:])

    # ======== raw phase: P2P broadcast + reduce + finish ========
    # (outside TileContext: the tile scheduler's single-core sim cannot
    # model remotely-incremented semaphores)  Every RAW edge — including
    # same-engine ones — is ordered through per-engine chain semaphores.
    nc.all_engine_barrier()

    # ---- all-to-all broadcast: slot j <- core (me XOR j) ----
    for j in range(M):
        rdests = [None] * M
        rdests[j] = (0, j)
        nc.gpsimd.remote_dma_broadcast(
            out_ap=recv[:, j, :],
            in_ap=part[:],
            remote_sem=recv_sem,
            local_sem=send_sem,
            rdests=rdests,
        ).then_inc(prep_sem, 1)
    nc.gpsimd.wait_ge(prep_sem, M)
    nc.gpsimd.trigger_dma(count=M)

    # ---- DVE: wait for all 8 slabs (8 senders x 16/8 units), reduce ----
    nc.vector.wait_ge(recv_sem, 16)
    nc.vector.tensor_tensor(
        out=s4[:], in0=recv[:, 0:4, :], in1=recv[:, 4:8, :], op=Alu.add
    ).then_inc(dvec, 1)
    nc.vector.wait_ge(dvec, 1)
    nc.vector.tensor_tensor(
        out=s2[:], in0=s4[:, 0:2, :], in1=s4[:, 2:4, :], op=Alu.add
    ).then_inc(dvec, 1)
    nc.vector.wait_ge(dvec, 2)
    nc.vector.tensor_tensor(
        out=bred[:], in0=s2[:, 0, :], in1=s2[:, 1, :], op=Alu.add
    ).then_inc(dvec, 1)  # dvec=3

    # ---- PE: four [64, RS] grid->ref matmuls (bred free-sliced) ----
    nc.tensor.wait_ge(dvec, 3)
    for ip, q, qsl in [
        (ip_ls, 0, slice(0, NI)),
        (ip_ns, 0, slice(NI, P)),
        (ip_lc, 1, slice(0, NI)),
        (ip_nq, 1, slice(NI, P)),
    ]:
        nc.tensor.matmul(
            ip[:], bred[:, qsl], kp_sb[:, q, :], start=True, stop=True
        ).then_inc(pec, 1)  # pec=1..4

    # ---- DVE: EPS corrections ----
    nc.vector.wait_ge(pec, 4)
    for ip, sbuf, cc in [
        (ip_ls, sb_ls, corr_col[0:NI, :]),
        (ip_ns, sb_ns, corr_col[NI:P, :]),
        (ip_lc, sb_lc, corr_col[0:NI, :]),
        (ip_nq, sb_nq, corr_col[NI:P, :]),
    ]:
        nc.vector.tensor_scalar(
            out=sbuf[:], in0=ip[:], scalar1=cc, scalar2=None, op0=Alu.add
        ).then_inc(dvec, 1)  # dvec=4..7

    # ---- PE: D rowsum + rho matmul ----
    nc.tensor.wait_ge(dvec, 7)
    nc.tensor.matmul(
        dps[:], ones_col[:], sb_ls[:], start=True, stop=True
    ).then_inc(pec, 1)  # pec=5
    nc.tensor.matmul(
        crp[:], rho_sb[:], sb_ns[:], start=True, stop=True
    ).then_inc(pec, 1)  # pec=6

    # ---- DVE: reciprocals + products ----
    nc.vector.wait_ge(dvec, 6)  # sb_lc written
    nc.vector.reciprocal_approx_fast(out=rec[:], in_=sb_lc[:]).then_inc(
        dvec, 1
    )  # dvec=8
    nc.vector.tensor_scalar(
        out=lam_out[:], in0=sb_ls[:], scalar1=1.0 / R, scalar2=None,
        op0=Alu.mult,
    ).then_inc(dvec, 1)  # dvec=9
    nc.vector.wait_ge(dvec, 8)  # rec (and sb_nq at 7) written
    nc.vector.tensor_mul(out=coarse[:], in0=sb_nq[:], in1=rec[:]).then_inc(
        dvec, 1
    )  # dvec=10
    nc.vector.wait_ge(pec, 5)  # dps written
    nc.vector.reciprocal_approx_fast(out=recd[:], in_=dps[:]).then_inc(
        dvec, 1
    )  # dvec=11

    # ---- PE: broadcast 1/D over 64 rows; ACT: copy to SBUF ----
    nc.tensor.wait_ge(dvec, 11)
    nc.tensor.matmul(
        dbp[:], ones_row[0:1, 0:NI], recd[0:1, :], start=True, stop=True
    ).then_inc(pec, 1)  # pec=7
    nc.scalar.wait_ge(pec, 7)
    nc.scalar.copy(out=dbc[:], in_=dbp[:]).then_inc(actc, 1)  # actc=1

    # ---- DVE: cross & transient; SYNC: output DMAs ----
    nc.vector.wait_ge(pec, 6)  # crp written (pec>=6)
    nc.vector.wait_ge(actc, 1)  # dbc written
    nc.vector.tensor_mul(out=cross[:], in0=crp[:], in1=dbc[:]).then_inc(
        dvec, 1
    )  # dvec=12
    nc.vector.wait_ge(dvec, 12)  # cross written (coarse at 10)
    nc.vector.tensor_sub(
        out=transient[:], in0=coarse[:], in1=cross[:]
    ).then_inc(dvec, 1)  # dvec=13

    nc.sync.wait_ge(dvec, 13)
    nc.sync.dma_start(out=out_t[0:NI, :], in_=lam_out[:]).then_inc(out_sem, 16)
    nc.sync.dma_start(out=out_t[NI : 2 * NI, :], in_=cross[:]).then_inc(
        out_sem, 16
    )
    nc.sync.dma_start(out=out_t[2 * NI : 3 * NI, :], in_=transient[:]).then_inc(
        out_sem, 16
    )
    nc.sync.wait_ge(out_sem, 48)

    # make sure our sends drained before program end
    nc.gpsimd.wait_ge(send_sem, 16 * M)

    nc.finalize()
    es.close()
    return nc


_prog_cache = {}


def _get_prog():
    if "p" not in _prog_cache:
        _prog_cache["p"] = build_program()
    return _prog_cache["p"]


last_results = None


def kernel(S, reference_timesteps, alpha, rho):
    global last_results
    import ml_dtypes

    S = np.ascontiguousarray(np.asarray(S, dtype=np.float32))
    ref = np.ascontiguousarray(
        np.asarray(reference_timesteps, dtype=np.float32)
    )
    rho = np.ascontiguousarray(np.asarray(rho, dtype=np.float32))
    a = float(np.asarray(alpha).reshape(-1)[0])

    assert S.shape == (N, 3) and ref.shape == (1, R) and rho.shape == (NI, NI)

    refd = ref[0].astype(np.float64)
    grid = GRID_LO + DG * np.arange(G)

    # deconvolved grid->ref kernels (Gaussian convolution identity)
    sig_s = 1.0 / np.sqrt(2.0 * a)
    sig_c = 1.0 / np.sqrt(2.0 * K_SCALE * a)

    def kp_mat(sig_k):
        sr = np.sqrt(sig_k * sig_k - SIG_V * SIG_V)
        A = DG * sig_k / (np.sqrt(2 * np.pi) * SIG_V * sr)
        return A * np.exp(
            -((refd[None, :] - grid[:, None]) ** 2) / (2 * sr * sr)
        )

    kp_s = kp_mat(sig_s)  # [G, R]
    kp_c = kp_mat(sig_c)

    nc = _get_prog()

    t = S[:, 0].astype(np.float64)
    v = S[:, 1].astype(np.float64)
    dims = S[:, 2].astype(np.int32)
    mask = (t > 0).astype(np.float64)
    cnt = np.bincount(dims, minlength=NI).astype(np.float64)
    sv = np.bincount(dims, weights=v * mask, minlength=NI)
    corr = np.concatenate([EPS * (cnt + 1.0), EPS * sv]).astype(np.float32)
    corr = corr.reshape(P, 1)

    # host-precomputed stationary weights [N, 128] in bf16
    comb = np.zeros((N, 2 * NI), np.float32)
    rows = np.arange(N)
    comb[rows, dims] = mask
    comb[rows, NI + dims] = mask * v
    comb = comb.astype(ml_dtypes.bfloat16)

    gb = np.ascontiguousarray(
        np.broadcast_to(grid.astype(np.float32)[None, :], (P, G))
    )

    in_maps = []
    for i in range(M):
        comb_i = comb[i * ND : (i + 1) * ND].reshape(NCHUNK, P, 2 * NI)
        comb_i = np.ascontiguousarray(comb_i.transpose(1, 0, 2))
        t_i = (
            S[i * ND : (i + 1) * ND, 0]
            .reshape(NCHUNK, P)
            .transpose(1, 0)
        )
        kp_i = np.stack(
            [
                kp_s[:, i * RS : (i + 1) * RS],
                kp_c[:, i * RS : (i + 1) * RS],
            ],
            axis=1,
        ).astype(ml_dtypes.bfloat16)
        in_maps.append(
            {
                "comb": comb_i,
                "gb": gb,
                "tsc": np.ascontiguousarray(t_i.astype(np.float32)),
                "rho": rho,
                "corr": corr,
                "kp": np.ascontiguousarray(kp_i),
            }
        )

    if os.environ.get("BASS_SIM"):
        import concourse.libnrt as libnrt
        from concourse.bass_interp import MultiCoreSim

        # fake_nrt has no driver NC-map ioctls; identity maps match the
        # sim's 8-cores-on-one-device model
        libnrt.get_trn2_nc_mapping = lambda: {
            (d, i): i for d in range(4) for i in range(8)
        }
        libnrt.get_device_id_to_routing_id_mapping = lambda: {
            d: d for d in range(4)
        }
        import concourse.bass_interp as _bi

        _bi.get_device_id_to_routing_id_mapping = (
            libnrt.get_device_id_to_routing_id_mapping
        )

        sim = MultiCoreSim(nc, M)
        for i in range(M):
            for k, val in in_maps[i].items():
                sim.cores[i].tensor(k)[:] = val
        sim.simulate()
        out = np.concatenate(
            [np.array(sim.cores[i].tensor("out")).T for i in range(M)], axis=0
        )
        last_results = None
    else:
        from concourse.bass_utils import run_bass_kernel_spmd

        tc_env = os.environ.get("BASS_TRACE_CORES")
        res = run_bass_kernel_spmd(
            nc,
            in_maps,
            list(range(M)),
            trace=bool(os.environ.get("BASS_TRACE")),
            trace_cores=(
                [int(x) for x in tc_env.split(",")] if tc_env else None
            ),
        )
        last_results = res
        out = np.concatenate(
            [np.asarray(res.results[i]["out"]).T for i in range(M)], axis=0
        )

    return np.ascontiguousarray(out).reshape(1, R, 3 * NI).astype(np.float32)


# revision 13
# speedup vs baseline: 49.0925x; 49.0925x over previous
"""Trainium2 Bass kernel for nn_Interpolator — Gaussian-scatter + P2P, v4.

Reference (N=32768 obs, R=2048 sorted ref timesteps, ninp=64, a=50):
    Ks[r,n] = exp(-a(ref_r - t_n)^2)*mask + EPS,  Kc same with 10a
    lam_s = Ks@onehot + EPS, num_s = Ks@(onehot*v), likewise coarse
    lam = lam_s/R; cross = (num_s@rho)/rowsum(lam_s); coarse = num_c/lam_c
    out = concat([lam, cross, coarse-cross], -1)   [1, R, 192]

Algorithm (NUFFT-style Gaussian gridding): scatter each observation onto a
uniform G=128 grid with a narrow Gaussian V (sigma_v = 1.5*dg), so
B_T[h,q] = sum_n V(h - t_n) * comb[n,q] accumulates ALL four segment sums
in one [128,128,128] matmul per 128-obs chunk (comb = [onehot*mask |
onehot*mask*v]).  Both kernels are then recovered exactly by grid-to-ref
matmuls with host-precomputed deconvolved Gaussians Kp (Gaussian*Gaussian
convolution identity; aliasing error ~e^-37).  No transposes needed:
the loop matmul directly produces the [h, q] layout the finish consumes.

Obs axis sharded 8 ways.  The partial B_T [128,128] bf16 (32 KB) is
combined across cores with remote_dma_broadcast (SBUF->SBUF P2P over
RMTV/D2D, SWDGE descriptors prepared after the loop, one trigger) instead
of a CC AllReduce — the ncfw first-call path cost ~55 us in the baseline.
Each core broadcasts its slab into slot j of every peer's receive buffer
(relative XOR addressing makes the same program valid on all 8 cores),
waits for recv_sem == 16 (8 senders x 2), then tree-adds the 8 slabs and
finishes only its own 256 ref columns: 2 grid matmuls, EPS correction,
reciprocals, rho matmul, writes [192, 256]; the host transposes and
concatenates the slices.
"""

import os
import sys

import numpy as np

sys.path.insert(0, "/opt/trn_rl_repo")

import concourse.bass as bass
import concourse.tile as tile
from concourse import bacc, mybir

# The image's antenv package lacks axon_hooks (NTFF profiling registry);
# register one so trace=True can profile HW exec time. Harmless if unused.
try:
    import antenv.axon_hooks  # noqa: F401
except ImportError:
    import types as _types

    _m = _types.ModuleType("antenv.axon_hooks")
    _m._hook = None

    def _set_hook(hook):
        _m._hook = hook

    def _get_hook():
        if _m._hook is None:
            try:
                from trn_agent_boot.trn_boot import _ntff_profile_via_ctypes

                _m._hook = _ntff_profile_via_ctypes("/opt/axon/libaxon_pjrt.so")
            except Exception:
                _m._hook = None
        return _m._hook

    _m.set_axon_ntff_profile_hook = _set_hook
    _m.get_axon_ntff_profile_hook = _get_hook
    sys.modules["antenv.axon_hooks"] = _m
    try:
        import antenv

        antenv.axon_hooks = _m
    except ImportError:
        pass

F32 = mybir.dt.float32
BF16 = mybir.dt.bfloat16
Alu = mybir.AluOpType
Act = mybir.ActivationFunctionType

N = 32768
R = 2048
NI = 64
M = 8
ND = N // M          # 4096 obs per core
P = 128
NCHUNK = ND // P     # 32
G = 128              # scatter grid points
RS = R // M          # 256 ref rows finished per core
EPS = 1e-7
K_SCALE = 10.0

GRID_LO = -0.05
GRID_HI = 1.05
DG = (GRID_HI - GRID_LO) / (G - 1)
SIG_V = 1.5 * DG
BV = 1.0 / (2.0 * SIG_V * SIG_V)


def build_program():
    nc = bacc.Bacc("TRN2")

    # host-reordered so every DMA is contiguous: comb[p, c, :] = row c*128+p
    comb_in = nc.declare_dram_parameter(
        "comb", [P, NCHUNK, 2 * NI], BF16, isOutput=False
    )
    gb_in = nc.declare_dram_parameter("gb", [P, G], F32, isOutput=False)
    tsc_in = nc.declare_dram_parameter("tsc", [P, NCHUNK], F32, isOutput=False)
    rho_in = nc.declare_dram_parameter("rho", [NI, NI], F32, isOutput=False)
    # corr[0:64] = EPS*(cnt+1); corr[64:128] = EPS*sv  (per-dim EPS pads)
    corr_in = nc.declare_dram_parameter("corr", [P, 1], F32, isOutput=False)
    # per-core deconvolved grid->ref kernels: [G, {smooth,coarse}, RS]
    kp_in = nc.declare_dram_parameter("kp", [G, 2, RS], BF16, isOutput=False)
    # output slice, quantity-major; host transposes to [RS, 192]
    out_t = nc.declare_dram_parameter("out", [3 * NI, RS], F32, isOutput=True)

    groups = [list(range(M))]

    with tile.TileContext(nc) as tc:
        with (
            tc.tile_pool(name="consts", bufs=1) as consts,
            tc.tile_pool(name="dram", bufs=1, space="DRAM") as dram,
        ):
            # warm up the collectives firmware with a 1-byte AllGather the
            # moment the kernel starts; its ~40us first-call ncfw wakeup then
            # overlaps the main loop, and the real AllGather (queued behind
            # it in CC program order) starts at its ~5us floor.
            warm_in = dram.tile([1, 1], mybir.dt.uint8, name="warm_in")
            warm_out = dram.tile(
                [M, 1], mybir.dt.uint8, name="warm_out", addr_space="Shared"
            )
            nc.gpsimd.collective_compute(
                "AllGather",
                Alu.bypass,
                replica_groups=groups,
                ins=[warm_in[:].opt()],
                outs=[warm_out[:].opt()],
            )

            # ---------------- constants ----------------
            gb = consts.tile([P, G], F32)
            nc.sync.dma_start(out=gb[:], in_=gb_in[:])
            tsc = consts.tile([P, NCHUNK], F32)
            nc.sync.dma_start(out=tsc[:], in_=tsc_in[:])
            combH = consts.tile([P, NCHUNK, 2 * NI], BF16)
            for q4 in range(4):
                cs = q4 * (NCHUNK // 4)
                ce = cs + NCHUNK // 4
                nc.sync.dma_start(
                    out=combH[:, cs:ce, :], in_=comb_in[:, cs:ce, :]
                )
            corr_col = consts.tile([P, 1], F32)
            nc.sync.dma_start(out=corr_col[:], in_=corr_in[:])
            rho_sb = consts.tile([NI, NI], F32)
            nc.sync.dma_start(out=rho_sb[:], in_=rho_in[:])
            kp_sb = consts.tile([G, 2, RS], BF16)
            nc.sync.dma_start(out=kp_sb[:], in_=kp_in[:])
            ones_row = consts.tile([1, P], F32)
            nc.vector.memset(ones_row, 1.0)
            ones_col = consts.tile([NI, 1], F32)
            nc.vector.memset(ones_col, 1.0)

            part = consts.tile([P, G], BF16)

            # ---------------- main loop ----------------
            with (
                tc.tile_pool(name="acc", bufs=1, space="PSUM") as accpool,
                tc.tile_pool(name="work", bufs=4) as work,
            ):
                acc = accpool.tile([P, G], F32, name="acc", tag="acc")

                for c in range(NCHUNK):
                    d = work.tile([P, G], F32, tag="d")
                    nc.vector.tensor_scalar(
                        out=d[:], in0=gb[:], scalar1=tsc[:, c : c + 1],
                        scalar2=None, op0=Alu.subtract,
                    )
                    d2 = work.tile([P, G], F32, tag="d2")
                    nc.gpsimd.tensor_tensor(
                        out=d2[:], in0=d[:], in1=d[:], op=Alu.mult
                    )
                    kek = work.tile([P, G], BF16, tag="kek")
                    nc.scalar.activation(
                        out=kek[:], in_=d2[:], func=Act.Exp, scale=-BV
                    )
                    nc.tensor.matmul(
                        acc[:, :],
                        kek[:, :],
                        combH[:, c, :],
                        start=(c == 0),
                        stop=(c == NCHUNK - 1),
                    )

                nc.vector.tensor_copy(out=part[:], in_=acc[:])

            # ---------------- AllGather + local tree-reduce ----------------
            ag_in = dram.tile([P, G], BF16, name="ag_in")
            ag_out = dram.tile([M, P, G], BF16, name="ag_out", addr_space="Shared")
            nc.sync.dma_start(out=ag_in[:], in_=part[:])
            nc.gpsimd.collective_compute(
                "AllGather",
                Alu.bypass,
                replica_groups=groups,
                ins=[ag_in[:].opt()],
                outs=[ag_out[:].opt()],
            )

            with (
                tc.tile_pool(name="fin", bufs=1) as fin,
                tc.tile_pool(name="fps", bufs=1, space="PSUM") as fps,
            ):
                recv = fin.tile([P, M, G], BF16)
                nc.sync.dma_start(
                    out=recv[:], in_=ag_out[:].rearrange("m p g -> p m g")
                )
                s4 = fin.tile([P, 4, G], BF16)
                nc.vector.tensor_tensor(
                    out=s4[:], in0=recv[:, 0:4, :], in1=recv[:, 4:8, :],
                    op=Alu.add,
                )
                s2 = fin.tile([P, 2, G], BF16)
                nc.vector.tensor_tensor(
                    out=s2[:], in0=s4[:, 0:2, :], in1=s4[:, 2:4, :], op=Alu.add
                )
                bred = fin.tile([P, G], BF16)
                nc.vector.tensor_tensor(
                    out=bred[:], in0=s2[:, 0, :], in1=s2[:, 1, :], op=Alu.add
                )

                # four [64, RS] grid->ref matmuls; bred free-sliced so every
                # PE output sits at partition base 0
                interp = {}
                for nm, q, qsl in [
                    ("ls", 0, slice(0, NI)),
                    ("ns", 0, slice(NI, P)),
                    ("lc", 1, slice(0, NI)),
                    ("nq", 1, slice(NI, P)),
                ]:
                    ip = fps.tile([NI, RS], F32, tag=f"ip_{nm}")
                    nc.tensor.matmul(
                        ip[:], bred[:, qsl], kp_sb[:, q, :],
                        start=True, stop=True,
                    )
                    sb = fin.tile([NI, RS], F32, name=f"sb_{nm}")
                    cc = (
                        corr_col[0:NI, :]
                        if nm in ("ls", "lc")
                        else corr_col[NI:P, :]
                    )
                    nc.vector.tensor_scalar(
                        out=sb[:], in0=ip[:], scalar1=cc, scalar2=None,
                        op0=Alu.add,
                    )
                    interp[nm] = sb

                ls, ns, lc, nq = (
                    interp[k] for k in ("ls", "ns", "lc", "nq")
                )

                # D[r] = sum_q ls[q, r] via PE; reciprocals for both ratios
                dps = fps.tile([1, RS], F32, tag="dps")
                nc.tensor.matmul(
                    dps[:], ones_col[:], ls[:], start=True, stop=True
                )
                rec = fin.tile([NI, RS], F32)
                nc.vector.reciprocal_approx_fast(out=rec[:], in_=lc[:])
                recd = fin.tile([1, RS], F32)
                nc.vector.reciprocal_approx_fast(out=recd[:], in_=dps[:])

                lam_out = fin.tile([NI, RS], F32)
                nc.vector.tensor_scalar(
                    out=lam_out[:], in0=ls[:], scalar1=1.0 / R, scalar2=None,
                    op0=Alu.mult,
                )
                coarse = fin.tile([NI, RS], F32)
                nc.vector.tensor_mul(out=coarse[:], in0=nq[:], in1=rec[:])

                crp = fps.tile([NI, RS], F32, tag="crp")
                nc.tensor.matmul(crp[:], rho_sb[:], ns[:], start=True, stop=True)
                dbp = fps.tile([NI, RS], F32, tag="dbp")
                nc.tensor.matmul(
                    dbp[:], ones_row[0:1, 0:NI], recd[0:1, :],
                    start=True, stop=True,
                )
                dbc = fin.tile([NI, RS], F32)
                nc.scalar.copy(out=dbc[:], in_=dbp[:])
                cross = fin.tile([NI, RS], F32)
                nc.vector.tensor_mul(out=cross[:], in0=crp[:], in1=dbc[:])
                transient = fin.tile([NI, RS], F32)
                nc.vector.tensor_sub(
                    out=transient[:], in0=coarse[:], in1=cross[:]
                )

                nc.sync.dma_start(out=out_t[0:NI, :], in_=lam_out[:])
                nc.sync.dma_start(out=out_t[NI : 2 * NI, :], in_=cross[:])
                nc.sync.dma_start(out=out_t[2 * NI : 3 * NI, :], in_=transient[:])

    nc.finalize()
    return nc


_prog_cache = {}


def _get_prog():
    if "p" not in _prog_cache:
        _prog_cache["p"] = build_program()
    return _prog_cache["p"]


last_results = None


def kernel(S, reference_timesteps, alpha, rho):
    global last_results
    import ml_dtypes

    S = np.ascontiguousarray(np.asarray(S, dtype=np.float32))
    ref = np.ascontiguousarray(
        np.asarray(reference_timesteps, dtype=np.float32)
    )
    rho = np.ascontiguousarray(np.asarray(rho, dtype=np.float32))
    a = float(np.asarray(alpha).reshape(-1)[0])

    assert S.shape == (N, 3) and ref.shape == (1, R) and rho.shape == (NI, NI)

    refd = ref[0].astype(np.float64)
    grid = GRID_LO + DG * np.arange(G)

    # deconvolved grid->ref kernels (Gaussian convolution identity)
    sig_s = 1.0 / np.sqrt(2.0 * a)
    sig_c = 1.0 / np.sqrt(2.0 * K_SCALE * a)

    def kp_mat(sig_k):
        sr = np.sqrt(sig_k * sig_k - SIG_V * SIG_V)
        A = DG * sig_k / (np.sqrt(2 * np.pi) * SIG_V * sr)
        return A * np.exp(
            -((refd[None, :] - grid[:, None]) ** 2) / (2 * sr * sr)
        )

    kp_s = kp_mat(sig_s)  # [G, R]
    kp_c = kp_mat(sig_c)

    nc = _get_prog()

    t = S[:, 0].astype(np.float64)
    v = S[:, 1].astype(np.float64)
    dims = S[:, 2].astype(np.int32)
    mask = (t > 0).astype(np.float64)
    cnt = np.bincount(dims, minlength=NI).astype(np.float64)
    sv = np.bincount(dims, weights=v * mask, minlength=NI)
    corr = np.concatenate([EPS * (cnt + 1.0), EPS * sv]).astype(np.float32)
    corr = corr.reshape(P, 1)

    # host-precomputed stationary weights [N, 128] in bf16
    comb = np.zeros((N, 2 * NI), np.float32)
    rows = np.arange(N)
    comb[rows, dims] = mask
    comb[rows, NI + dims] = mask * v
    comb = comb.astype(ml_dtypes.bfloat16)

    gb = np.ascontiguousarray(
        np.broadcast_to(grid.astype(np.float32)[None, :], (P, G))
    )

    in_maps = []
    for i in range(M):
        comb_i = comb[i * ND : (i + 1) * ND].reshape(NCHUNK, P, 2 * NI)
        comb_i = np.ascontiguousarray(comb_i.transpose(1, 0, 2))
        t_i = (
            S[i * ND : (i + 1) * ND, 0]
            .reshape(NCHUNK, P)
            .transpose(1, 0)
        )
        kp_i = np.stack(
            [
                kp_s[:, i * RS : (i + 1) * RS],
                kp_c[:, i * RS : (i + 1) * RS],
            ],
            axis=1,
        ).astype(ml_dtypes.bfloat16)
        in_maps.append(
            {
                "comb": comb_i,
                "gb": gb,
                "tsc": np.ascontiguousarray(t_i.astype(np.float32)),
                "rho": rho,
                "corr": corr,
                "kp": np.ascontiguousarray(kp_i),
            }
        )

    if os.environ.get("BASS_SIM"):
        import concourse.libnrt as libnrt
        from concourse.bass_interp import MultiCoreSim

        # fake_nrt has no driver NC-map ioctls; identity maps match the
        # sim's 8-cores-on-one-device model
        libnrt.get_trn2_nc_mapping = lambda: {
            (d, i): i for d in range(4) for i in range(8)
        }
        libnrt.get_device_id_to_routing_id_mapping = lambda: {
            d: d for d in range(4)
        }
        import concourse.bass_interp as _bi

        _bi.get_device_id_to_routing_id_mapping = (
            libnrt.get_device_id_to_routing_id_mapping
        )

        sim = MultiCoreSim(nc, M)
        for i in range(M):
            for k, val in in_maps[i].items():
                sim.cores[i].tensor(k)[:] = val
        sim.simulate()
        out = np.concatenate(
            [np.array(sim.cores[i].tensor("out")).T for i in range(M)], axis=0
        )
        last_results = None
    else:
        from concourse.bass_utils import run_bass_kernel_spmd

        tc_env = os.environ.get("BASS_TRACE_CORES")
        res = run_bass_kernel_spmd(
            nc,
            in_maps,
            list(range(M)),
            trace=bool(os.environ.get("BASS_TRACE")),
            trace_cores=(
                [int(x) for x in tc_env.split(",")] if tc_env else None
            ),
        )
        last_results = res
        out = np.concatenate(
            [np.asarray(res.results[i]["out"]).T for i in range(M)], axis=0
        )

    return np.ascontiguousarray(out).reshape(1, R, 3 * NI).astype(np.float32)
